# revision 9
# baseline (speedup 1.0000x reference)
"""Trainium2 Bass kernel: mixture-of-Gaussians mean log-likelihood.

Computes mean_n logsumexp_k [ -0.5*quad(n,k) + c_k ] over N=2M points,
K=32 components, D=16 dims, data-parallel over 8 NeuronCores.

Shared-basis quadratic form with two-point PE-column packing:
  quad'_k(n) = x~^T Q'_k x~ ~= sum_{m=1}^{64} beta_km (v_m^T x~)^2
  (R=64 shared unit vectors fit on host by VarPro/L-BFGS; residual biases
  the answer by ~-0.057 absolute = 2.7e-3 relative, well inside 2e-2.)

Because the contraction is only 17-deep, TWO points ride in each PE
column: rhs rows 0-16 hold point a's x~, rows 17-33 point b's; the
stationary [34, 128] holds V in rows 0-16 for output columns 0-63 and V
in rows 17-33 for columns 64-127.  One matmul column thus yields 64
features for each of 2 points -> every downstream free-size cost
(square, PSUM escape) is halved vs one-point-per-column.

Per 2048-point sub-chunk (1024 pair-columns):
  T = Vbig^T x~pair   2 matmuls of 512 cols      (PE -> PSUM [128,1024] f32)
  Phi = T^2  fp16     split: ScalarE Square head | DVE copy + Pool mul tail
  quad = Phi^T beta   16 matmuls: lhsT = Phi[slot, 128 cols], rhs = beta
                      (slot A rows 0:64, slot B rows 64:128) -> [128pts, 32]
  e = exp(-quad/2)    one batched activation, f32 out (fp16 would underflow)
  s = sum_k e         Pool tensor_reduce over innermost 32
  log s               single batched Ln + accumulate at the end
"""

from contextlib import ExitStack

import numpy as np

import concourse.bass as bass
import concourse.mybir as mybir
import concourse.tile as tile
from concourse import bacc
from concourse.bass_utils import run_bass_kernel_spmd

F32 = mybir.dt.float32
F16 = mybir.dt.float16

# Problem constants
N_TOTAL = 2_000_000
D = 16
K = 32
NCORES = 8
NC = N_TOTAL // NCORES   # 250_000 points per core
P = D + 1                # 17: features + ones row
R = 64                   # shared quadratic basis size (per point)
LOG_2PI = float(np.log(2.0 * np.pi))
SHIFT = 23.0             # folded into Q' so log s lands near 0

# Tiling (per core), in pair-columns (each column = 2 points)
NPC2 = 126_976           # pair-cols; NPC = 253_952 points, 3952 pads
NPC = 2 * NPC2
WCHUNK = 4096            # DMA chunk pair-cols; 31 chunks
SUB = 1024               # compute sub-chunk pair-cols (2048 points)
NTILES = NPC // 128      # 1984 point-tiles -> s_buf columns
TPS = 2 * SUB // 128     # point-tiles per sub-chunk = 16
SQ_SCALAR = 384          # pair-cols of each SUB squared on ScalarE

_MODULE_CACHE: dict = {}
_FIT_CACHE: dict = {}


def build_module(reps: int = 1, sq_scalar: int = SQ_SCALAR):
    """Device I/O (per core):
      t    [34, NPC2]  f16  input  (two x~^T stacks; rows 16/33 = ones)
      v    [34, 128]   f16  input  (block-diag stationary: V at rows 0-16
                                    cols 0-63, V at rows 17-33 cols 64-127)
      beta [128, 64]   f16  input  (cols 0-31: [beta;0] for slot A,
                                    cols 32-63: [0;beta] for slot B)
      out  [128, 1]    f32  output (sum over tiles of log s)
    """
    assert NPC2 % WCHUNK == 0 and WCHUNK % SUB == 0
    nchunks = NPC2 // WCHUNK
    subs_per_chunk = WCHUNK // SUB
    pair_tiles = SUB // 128          # 8 pair-tiles -> 16 point-tiles

    nc = bacc.Bacc("TRN2", target_bir_lowering=False, debug=False)

    t_in = nc.dram_tensor("t", [2 * P, NPC2], F16, kind="ExternalInput").ap()
    v_in = nc.dram_tensor("v", [2 * P, 128], F16, kind="ExternalInput").ap()
    b_in = nc.dram_tensor("beta", [128, 2 * K], F16, kind="ExternalInput").ap()
    out = nc.dram_tensor("out", [128, 1], F32, kind="ExternalOutput").ap()

    AX = mybir.AxisListType
    OP = mybir.AluOpType
    AF = mybir.ActivationFunctionType

    with tile.TileContext(nc) as tc, ExitStack() as ctx:
        data_pool = ctx.enter_context(tc.tile_pool(name="data", bufs=2))
        tpool = ctx.enter_context(tc.tile_pool(name="tps", bufs=2, space="PSUM"))
        qpool = ctx.enter_context(tc.tile_pool(name="qps", bufs=2, space="PSUM"))
        phipool = ctx.enter_context(tc.tile_pool(name="phi", bufs=2))
        tcpool = ctx.enter_context(tc.tile_pool(name="tc16", bufs=2))
        epool = ctx.enter_context(tc.tile_pool(name="eb", bufs=2))
        cpool = ctx.enter_context(tc.tile_pool(name="const", bufs=1))

        vt = cpool.tile([2 * P, 128], F16)
        nc.sync.dma_start(vt[:], v_in)
        bt = cpool.tile([128, 2 * K], F16)
        nc.sync.dma_start(bt[:], b_in)

        s_buf = cpool.tile([128, NTILES], F16)
        ln_buf = cpool.tile([128, NTILES], F32)
        res = cpool.tile([128, 1], F32)

        def emit_B(prev_ph):
            """PE stage B for the previous sub (queued behind this sub's A
            so the PE queue never stalls waiting on squares)."""
            qd = qpool.tile([128, pair_tiles, 2 * K], F32, tag="qd")
            for j in range(pair_tiles):
                nc.tensor.matmul(qd[:, j],
                                 prev_ph[:, j * 128:(j + 1) * 128],
                                 bt[:], start=True, stop=True)
            return qd

        def emit_tail(prev_g, qd):
            """exp + ksum for the previous sub: queued after this sub's
            sq/copy on ScalarE/DVE to avoid head-of-line blocking."""
            eb = epool.tile([128, pair_tiles, 2 * K], F16, tag="eb")
            nc.scalar.activation(eb[:], qd[:], AF.Exp, scale=-0.5)
            eb4 = eb[:].rearrange("p t (s k) -> p t s k", s=2)
            with nc.allow_low_precision("fp16 s; end-clamped"):
                nc.vector.tensor_reduce(
                    s_buf[:, prev_g * TPS:(prev_g + 1) * TPS].rearrange(
                        "p (t s) -> p t s", s=2),
                    eb4, axis=AX.X, op=OP.add)

        def emit_main():
            prev = None
            for ch in range(nchunks):
                dt = data_pool.tile([2 * P, WCHUNK], F16, tag="dt")
                nc.sync.dma_start(
                    dt[:], t_in[:, ch * WCHUNK:(ch + 1) * WCHUNK])
                for s in range(subs_per_chunk):
                    g = ch * subs_per_chunk + s
                    xs = dt[:, s * SUB:(s + 1) * SUB]
                    tt = tpool.tile([128, SUB], F32, tag="T")
                    for h in range(SUB // 512):
                        nc.tensor.matmul(tt[:, h * 512:(h + 1) * 512],
                                         vt[:], xs[:, h * 512:(h + 1) * 512],
                                         start=True, stop=True)
                    qd_prev = emit_B(prev[1]) if prev is not None else None
                    ph = phipool.tile([128, SUB], F16, tag="ph")
                    # square split: ScalarE head, DVE-copy + Pool-mul tail
                    if sq_scalar > 0:
                        nc.scalar.activation(ph[:, 0:sq_scalar],
                                             tt[:, 0:sq_scalar], AF.Square)
                    if sq_scalar < SUB:
                        tc16 = tcpool.tile([128, SUB - sq_scalar], F16,
                                           tag="tc")
                        nc.vector.tensor_copy(tc16[:], tt[:, sq_scalar:SUB])
                        nc.gpsimd.tensor_mul(ph[:, sq_scalar:SUB],
                                             tc16[:], tc16[:])
                    if prev is not None:
                        emit_tail(prev[0], qd_prev)
                    prev = (g, ph)
            qd_prev = emit_B(prev[1])
            emit_tail(prev[0], qd_prev)

        if reps == 1:
            emit_main()
        else:
            with tc.For_i(0, reps, 1):
                emit_main()

        # rescue fp16-underflowed s (few points per ~1e5): ln(6e-8) ~ -16.6
        nc.gpsimd.tensor_scalar_max(s_buf[:], s_buf[:], 6e-8)
        nc.scalar.activation(ln_buf[:], s_buf[:], AF.Ln,
                             accum_out=res[:, 0:1])
        nc.sync.dma_start(out, res[:])

    if not nc.is_finalized():
        nc.finalize()
    return nc


def _fit_shared_basis(Q):
    """V [17,R], beta [R,K] with Q_k ~= sum_m beta_km v_m v_m^T (VarPro)."""
    from scipy.optimize import minimize

    Kk = Q.shape[0]
    Qt = Q.reshape(Kk, P * P).T

    def obj(vflat):
        V = vflat.reshape(P, R)
        F = np.einsum('pm,qm->mpq', V, V).reshape(R, P * P)
        beta, *_ = np.linalg.lstsq(F.T, Qt, rcond=None)
        E = Qt - F.T @ beta
        Emat = E.T.reshape(Kk, P, P)
        G = -2 * np.einsum('kpq,qm,mk->pm', Emat, V, beta)
        return 0.5 * np.sum(E * E), G.ravel()

    rng = np.random.default_rng(11)
    best = None
    for _ in range(3):
        r = minimize(obj, rng.standard_normal(P * R), jac=True,
                     method='L-BFGS-B',
                     options=dict(maxiter=6000, maxfun=18000,
                                  ftol=1e-20, gtol=1e-16))
        if best is None or r.fun < best[0]:
            best = (r.fun, r.x)
    rel = np.sqrt(2 * best[0] / np.sum(Q * Q))
    assert rel < 0.15, f"shared-basis fit failed: rel={rel:.2e}"

    V = best[1].reshape(P, R)
    V /= np.linalg.norm(V, axis=0)
    Vq = V.astype(np.float16).astype(np.float64)
    Fq = np.einsum('pm,qm->mpq', Vq, Vq).reshape(R, P * P)
    beta, *_ = np.linalg.lstsq(Fq.T, Qt, rcond=None)
    return Vq, beta


def host_params(means, cov_parts, log_weights):
    """-> vbig [34,128] f16, beta_dup [128,32] f16, logs0 (pad log s)."""
    A = np.asarray(cov_parts, dtype=np.float64)
    mu = np.asarray(means, dtype=np.float64)
    w = np.asarray(log_weights, dtype=np.float64)

    key = (A.tobytes(), mu.tobytes(), w.tobytes())
    if key in _FIT_CACHE:
        return _FIT_CACHE[key]

    cov = np.einsum('kij,klj->kil', A, A)
    L = np.linalg.cholesky(cov)
    eye = np.eye(D, dtype=np.float64)
    M = np.stack([np.linalg.solve(L[k], eye) for k in range(K)])
    b = np.einsum('kij,kj->ki', M, mu)
    logdet = np.log(np.diagonal(L, axis1=1, axis2=2)).sum(axis=1)
    c = -0.5 * D * LOG_2PI - logdet + w ** 2

    Q = np.zeros((K, P, P))
    for k in range(K):
        Q[k, :D, :D] = M[k].T @ M[k]
        Q[k, :D, D] = -M[k].T @ b[k]
        Q[k, D, :D] = -M[k].T @ b[k]
        Q[k, D, D] = b[k] @ b[k] - 2.0 * (c[k] + SHIFT)

    Vq, beta = _fit_shared_basis(Q)
    V16 = Vq.astype(np.float16)                       # [17, 64]
    beta16 = beta.astype(np.float16)                  # [64, 32]

    vbig = np.zeros((2 * P, 128), dtype=np.float16)
    vbig[0:P, 0:R] = V16
    vbig[P:2 * P, R:128] = V16
    beta_dup = np.zeros((128, 2 * K), dtype=np.float16)
    beta_dup[0:R, 0:K] = beta16       # slot A reads Phi rows 0-63
    beta_dup[R:128, K:2 * K] = beta16  # slot B reads Phi rows 64-127

    T_pad = Vq[D, :].astype(np.float16).astype(np.float64)
    phi_pad = (T_pad * T_pad).astype(np.float16).astype(np.float64)
    quad_pad = phi_pad @ beta16.astype(np.float64)
    logs0 = float(np.log(np.exp(-0.5 * quad_pad).sum()))

    out = (np.ascontiguousarray(vbig), np.ascontiguousarray(beta_dup), logs0)
    _FIT_CACHE[key] = out
    return out


def build_t(data_core: np.ndarray) -> np.ndarray:
    """[npts, 16] f32 -> [34, NPC2] f16: slot A rows 0-16, slot B 17-33."""
    npts = data_core.shape[0]
    x = np.zeros((NPC, D), dtype=np.float16)
    x[:npts] = data_core.astype(np.float16)
    t = np.empty((2 * P, NPC2), dtype=np.float16)
    t[0:D] = x[:NPC2].T
    t[D] = 1.0
    t[P:P + D] = x[NPC2:].T
    t[P + D] = 1.0
    return np.ascontiguousarray(t)


def _get_module():
    if "m" not in _MODULE_CACHE:
        _MODULE_CACHE["m"] = build_module()
    return _MODULE_CACHE["m"]


def run(data, means, cov_parts, log_weights, trace=False, **trace_kwargs):
    """Run on 8 cores; returns (answer_scalar, BassKernelResults)."""
    data = np.asarray(data)
    assert data.shape == (N_TOTAL, D), data.shape
    nc = _get_module()
    vbig, beta_dup, logs0 = host_params(means, cov_parts, log_weights)

    in_maps = []
    for core in range(NCORES):
        shard = data[core * NC:(core + 1) * NC]
        in_maps.append({"t": build_t(shard), "v": vbig, "beta": beta_dup})
    res = run_bass_kernel_spmd(nc, in_maps, core_ids=list(range(NCORES)),
                               trace=trace, **trace_kwargs)

    total = 0.0
    for r in res.results:
        total += r["out"].astype(np.float64).sum()
    npad = NCORES * (NPC - NC)
    answer = (total - npad * logs0 - N_TOTAL * SHIFT) / N_TOTAL
    return np.float32(answer), res


def kernel(data, means, cov_parts, log_weights):
    ans, _ = run(data, means, cov_parts, log_weights, trace=False)
    return ans
